# revision 1
# baseline (speedup 1.0000x reference)
"""AdaLN-modulated multi-head attention block on 8 TRN2 NeuronCores.

Shapes (hardcoded): B=8, T=1024, D=1024, H=16 heads, e=64 head dim.
Sharding: pure data-parallel - one batch element per core, weights
replicated, no collectives.

Per-core pipeline ("T" suffix = feature-major [feature, token] layout):
  0. mod = silu(time) @ mod_w + mod_b; broadcast shift/scale+1/gate
  1. h = LN(x)*(1+scale)+shift (token-major, bn_stats); hT = h.T (PE)
  2. qT,kT = (h @ w_qkv[:, :2048]).T   (w_qkv stationary, f32r)
     v     = h @ w_qkv[:, 2048:]       (hT stationary), with a ones
             column per head so attn@v also yields softmax denominators
  3. per-head LN + RoPE on qT,kT in feature-major layout
     (segment-matmul stats, broadcast matmuls, pair-swap matmul)
  4. per head: scoresT = kT.T @ qT (bf16); exp on ACT (scale=1/8);
     oT = [v|1].T @ expT; normalize rows by broadcast reciprocal;
     result overwrites qT storage (head rows are dead by then)
  5. y = (oT.T @ w_out) * gate  (bf16 out-proj)
"""

import sys

try:
    import concourse  # noqa: F401  (provided by the environment, e.g. axon_site)
except ImportError:
    sys.path.append("/opt/trn_rl_repo")

import contextlib

import numpy as np

import concourse.bass as bass
import concourse.mybir as mybir
import concourse.tile as tile
from concourse import bacc
from concourse.bass_utils import run_bass_kernel_spmd

F32 = mybir.dt.float32
F32R = mybir.dt.float32r
BF16 = mybir.dt.bfloat16
AF = mybir.ActivationFunctionType
OP = mybir.AluOpType

B, T, D, TD = 8, 1024, 1024, 1024
H, E = 16, 64
P = 128
NT = T // P          # 8 token tiles
ND = D // P          # 8 feature tiles
EPS = 1e-6
N3 = 3 * D


def r(ap):
    """View an f32 AP as float32r for full-rate TensorE matmuls."""
    return ap.bitcast(F32R)


def build_nc(apply_qk_weight: bool):
    nc = bacc.Bacc("TRN2", target_bir_lowering=False, debug=False, num_devices=8)

    aps = {}
    aps["x"] = nc.dram_tensor("x", [T, D], F32, kind="ExternalInput").ap()
    aps["time"] = nc.dram_tensor("time", [TD], F32, kind="ExternalInput").ap()
    aps["mod_w"] = nc.dram_tensor("mod_w", [TD, N3], F32, kind="ExternalInput").ap()
    aps["mod_b"] = nc.dram_tensor("mod_b", [N3], F32, kind="ExternalInput").ap()
    aps["w_qkv"] = nc.dram_tensor("w_qkv", [D, N3], F32, kind="ExternalInput").ap()
    aps["w_out"] = nc.dram_tensor("w_out", [D, D], F32, kind="ExternalInput").ap()
    # host-precomputed constants
    aps["cs_full"] = nc.dram_tensor("cs_full", [P, T], BF16, kind="ExternalInput").ap()
    aps["sn_full"] = nc.dram_tensor("sn_full", [P, T], BF16, kind="ExternalInput").ap()
    aps["eseg"] = nc.dram_tensor("eseg", [P, ND, 16], BF16, kind="ExternalInput").ap()
    aps["bseg"] = nc.dram_tensor("bseg", [16, ND, P], BF16, kind="ExternalInput").ap()
    aps["pswap"] = nc.dram_tensor("pswap", [P, P], BF16, kind="ExternalInput").ap()
    aps["ident"] = nc.dram_tensor("ident", [P, P], BF16, kind="ExternalInput").ap()
    aps["ones_row"] = nc.dram_tensor("ones_row", [1, P], BF16, kind="ExternalInput").ap()
    aps["wq_col"] = nc.dram_tensor("wq_col", [P, 1], F32, kind="ExternalInput").ap()
    aps["wk_col"] = nc.dram_tensor("wk_col", [P, 1], F32, kind="ExternalInput").ap()

    aps["out"] = nc.dram_tensor("out", [T, D], F32, kind="ExternalOutput").ap()

    with tile.TileContext(nc) as tc:
        _body(nc, tc, aps, apply_qk_weight)
    nc.finalize()
    return nc


def _body(nc, tc, aps, apply_qk_weight):
    x_e, time_e, modw_e = aps["x"], aps["time"], aps["mod_w"]
    modb_e, wqkv_e, wout_e = aps["mod_b"], aps["w_qkv"], aps["w_out"]
    out_e = aps["out"]

    ctx = contextlib.ExitStack()
    with ctx:
        consts = ctx.enter_context(tc.tile_pool(name="consts", bufs=1))
        big = ctx.enter_context(tc.tile_pool(name="big", bufs=1))
        wstr = ctx.enter_context(tc.tile_pool(name="wstr", bufs=1))
        temps = ctx.enter_context(tc.tile_pool(name="temps", bufs=2))
        small = ctx.enter_context(tc.tile_pool(name="small", bufs=1))
        psum = ctx.enter_context(tc.tile_pool(name="psum", bufs=2, space="PSUM"))

        def psA(shape, name):
            return psum.tile(shape, F32, tag="psA", bufs=2, name=name,
                             padded_shape=[P, 1024])

        def psB(shape, name):
            return psum.tile(shape, F32, tag="psB", bufs=4, name=name,
                             padded_shape=[P, 512])

        # ---- constants into SBUF -------------------------------------
        def cload(key, shape, dtype, name):
            t = consts.tile(shape, dtype, tag=name, name=name)
            nc.sync.dma_start(t[:], aps[key])
            return t

        cs_sb = cload("cs_full", [P, T], BF16, "cs_sb")
        sn_sb = cload("sn_full", [P, T], BF16, "sn_sb")
        eseg_sb = cload("eseg", [P, ND, 16], BF16, "eseg_sb")
        bseg_sb = cload("bseg", [16, ND, P], BF16, "bseg_sb")
        pswap_sb = cload("pswap", [P, P], BF16, "pswap_sb")
        ident_sb = cload("ident", [P, P], BF16, "ident_sb")
        ones_sb = cload("ones_row", [1, P], BF16, "ones_sb")
        wq_sb = cload("wq_col", [P, 1], F32, "wq_sb")
        wk_sb = cload("wk_col", [P, 1], F32, "wk_sb")
        eps_sb = consts.tile([P, 1], F32, tag="eps_sb", name="eps_sb")
        nc.vector.memset(eps_sb[:], EPS)

        # ---- big resident tensors ------------------------------------
        hT = big.tile([P, ND, T], BF16, tag="hT", name="hT")       # 16K/part
        qT = big.tile([P, ND, T], BF16, tag="qT", name="qT")       # 16K
        kT = big.tile([P, ND, T], BF16, tag="kT", name="kT")       # 16K
        v_sb = big.tile([P, NT, H, E + 16], BF16, tag="v", name="v_sb")  # 20K
        oTn = qT   # head rows of qT are dead once that head's scores ran

        # ==============================================================
        # Stage 0: mod = silu(time) @ mod_w + mod_b
        # ==============================================================
        t8 = small.tile([P, TD // P], F32, tag="t8", name="t8")
        nc.sync.dma_start(t8[:], time_e.rearrange("(o p) -> p o", p=P))
        sig8 = small.tile([P, TD // P], F32, tag="sig8", name="sig8")
        nc.scalar.activation(sig8[:], t8[:], AF.Sigmoid)
        silu8 = small.tile([P, TD // P], BF16, tag="silu8", name="silu8")
        nc.vector.tensor_mul(silu8[:], sig8[:], t8[:])

        # three groups: shift, scale, gate - each [1, 1024], staged then
        # broadcast to [128, 1024]
        bc_dst = {}
        for g, gname in enumerate(("shiftB", "scale1B", "gateB")):
            mrowf = temps.tile([1, D], F32, tag="rbc", name=f"mrowf{g}")
            nc.sync.dma_start(mrowf[:], modb_e[None, g * D:(g + 1) * D])
            mrow = small.tile([1, D], BF16, tag="mrow", bufs=1, name=f"mrow{g}")
            for n2 in range(2):
                col0 = g * D + n2 * 512
                mp = psB([1, 512], f"modp{g}_{n2}")
                for kc in range(TD // P):
                    mwf = wstr.tile([P, 512], F32, tag="modwf", bufs=6,
                                    name=f"mwf{g}_{n2}_{kc}")
                    nc.sync.dma_start(mwf[:], modw_e[kc * P:(kc + 1) * P,
                                                     col0:col0 + 512])
                    mw = wstr.tile([P, 512], BF16, tag="modw", bufs=6,
                                   name=f"mw{g}_{n2}_{kc}")
                    nc.scalar.copy(mw[:], mwf[:])
                    nc.tensor.matmul(mp[:], silu8[:, kc:kc + 1], mw[:],
                                     start=(kc == 0), stop=(kc == TD // P - 1))
                # mrowf holds the bias slice; add the matmul result (bf16 out)
                sl = slice(n2 * 512, (n2 + 1) * 512)
                nc.vector.tensor_add(mrow[:, sl], mrowf[:, sl], mp[:])
            if gname == "scale1B":
                nc.vector.tensor_scalar_add(mrow[:], mrow[:], 1.0)
            if gname == "gateB":
                dst = consts.tile([P, D], BF16, tag=gname, name=gname)
                for n2 in range(2):
                    sl = slice(n2 * 512, (n2 + 1) * 512)
                    bp = psA([P, 512], f"bc{g}_{n2}")
                    nc.tensor.matmul(bp[:], ones_sb[:], mrow[:, sl])
                    nc.vector.tensor_copy(dst[:, sl], bp[:])
                bc_dst[gname] = dst
            else:
                # transpose the [1, D] row into per-feature columns [P, ND]
                col = consts.tile([P, ND], F32, tag=f"col{g}", name=f"col{g}")
                cp = psum.tile([P, 2 * ND], BF16, tag="psB", bufs=4,
                               name=f"colp{g}", padded_shape=[P, 512])
                for dc in range(ND):
                    nc.tensor.transpose(cp[:, 2 * dc:2 * dc + 1],
                                        mrow[:, dc * P:(dc + 1) * P],
                                        ident_sb[0:1, 0:1])
                nc.vector.tensor_copy(col[:], cp[:].rearrange(
                    "p (d two) -> p d two", two=2)[:, :, 0])
                bc_dst[gname] = col
        shcol, sc1col, gateB = bc_dst["shiftB"], bc_dst["scale1B"], bc_dst["gateB"]

        # ==============================================================
        # Stage 1: h = LN(x)*(scale+1)+shift ; hT = h.T
        # ==============================================================
        for tt in range(NT):
            xt = temps.tile([P, D], F32, tag="xt", name=f"xt{tt}")
            nc.sync.dma_start(xt[:], x_e[tt * P:(tt + 1) * P, :])
            st = small.tile([P, 2, 6], F32, tag="bnst", bufs=2, name=f"st{tt}")
            nc.vector.bn_stats(st[:, 0, :], xt[:, 0:512])
            nc.vector.bn_stats(st[:, 1, :], xt[:, 512:1024])
            mv = small.tile([P, 2], F32, tag="bnmv", bufs=2, name=f"mv{tt}")
            nc.vector.bn_aggr(mv[:], st[:])
            sd = small.tile([P, 1], F32, tag="sd", bufs=2, name=f"sd{tt}")
            nc.scalar.activation(sd[:], mv[:, 1:2], AF.Sqrt, bias=eps_sb[:])
            rstd = small.tile([P, 1], F32, tag="rstd", bufs=2, name=f"rstd{tt}")
            nc.vector.reciprocal(rstd[:], sd[:])
            nmr = small.tile([P, 1], F32, tag="nmr", bufs=2, name=f"nmr{tt}")
            nc.vector.tensor_mul(nmr[:], mv[:, 0:1], rstd[:])
            nc.vector.tensor_scalar_mul(nmr[:], nmr[:], -1.0)

            xn = temps.tile([P, D], BF16, tag="xnb", name=f"xn{tt}")
            nc.vector.tensor_scalar(xn[:], xt[:], rstd[:], nmr[:], OP.mult, OP.add)
            for dc in range(ND):
                tp = psum.tile([P, P], BF16, tag="psA", bufs=2,
                               name=f"tr{tt}_{dc}", padded_shape=[P, 1024])
                nc.tensor.transpose(tp[:], xn[:, dc * P:(dc + 1) * P], ident_sb[:])
                nc.vector.tensor_scalar(hT[:, dc, tt * P:(tt + 1) * P], tp[:],
                                        sc1col[:, dc:dc + 1], shcol[:, dc:dc + 1],
                                        OP.mult, OP.add)

        # ==============================================================
        # Stage 2a: v = h @ w_qkv[:, 2048:] (+ ones col per head)
        # ==============================================================
        nc.vector.memset(v_sb[:, :, :, E:E + 16], 0.0)
        for h in range(H):
            nc.vector.memset(v_sb[:, :, h, E + h], 1.0)
        for nv in range(2):          # 2 blocks of 512 cols
            wv_tiles = []
            for kc in range(ND):
                wvf = wstr.tile([P, 512], F32, tag="wvf", bufs=2,
                                name=f"wvf{nv}_{kc}")
                nc.sync.dma_start(wvf[:], wqkv_e[kc * P:(kc + 1) * P,
                                                 2048 + nv * 512: 2048 + (nv + 1) * 512])
                wv = wstr.tile([P, 512], BF16, tag=f"wv{kc}", name=f"wv{nv}_{kc}")
                nc.scalar.copy(wv[:], wvf[:])
                wv_tiles.append(wv)
            for tt in range(NT):
                vp = psA([P, 512], f"vp{nv}_{tt}")
                for kc in range(ND):
                    nc.tensor.matmul(vp[:], hT[:, kc, tt * P:(tt + 1) * P],
                                     wv_tiles[kc][:],
                                     start=(kc == 0), stop=(kc == ND - 1))
                nc.vector.tensor_copy(
                    v_sb[:, tt, nv * 8:(nv + 1) * 8, 0:E],
                    vp[:].rearrange("p (h e) -> p h e", e=E))

        # ==============================================================
        # Stage 2b: qT / kT (feature-major, bf16)
        # ==============================================================
        for jc in range(2 * ND):
            dst = qT if jc < ND else kT
            jd = jc % ND
            wq_tiles = []
            for kc in range(ND):
                wtf = wstr.tile([P, P], F32, tag="wqkf", bufs=2,
                                name=f"wqkf{jc}_{kc}")
                nc.sync.dma_start(wtf[:], wqkv_e[kc * P:(kc + 1) * P,
                                                 jc * P:(jc + 1) * P])
                wt = wstr.tile([P, P], BF16, tag=f"wqk{kc}", bufs=2,
                               name=f"wqk{jc}_{kc}")
                nc.scalar.copy(wt[:], wtf[:])
                wq_tiles.append(wt)
            for tn in range(2):
                qp = psA([P, 512], f"qp{jc}_{tn}")
                for kc in range(ND):
                    nc.tensor.matmul(qp[:], wq_tiles[kc][:],
                                     hT[:, kc, tn * 512:(tn + 1) * 512],
                                     start=(kc == 0), stop=(kc == ND - 1))
                nc.vector.tensor_copy(dst[:, jd, tn * 512:(tn + 1) * 512], qp[:])

        # ==============================================================
        # Stage 3: per-head LN + RoPE on qT and kT (in place)
        # stats for both tensors first, then q/k applies interleaved
        # per tile so stage 4's head pairs unlock progressively
        # ==============================================================
        AB = {}
        for which, src_t in ((0, qT), (1, kT)):
            sum_ps = [psB([16, 512], f"sum{which}_{tn}") for tn in range(2)]
            ssq_ps = [psB([16, 512], f"ssq{which}_{tn}") for tn in range(2)]
            for jt in range(ND):
                sq = temps.tile([P, T], BF16, tag="sqt", bufs=1, name=f"sq{which}_{jt}")
                nc.scalar.activation(sq[:], src_t[:, jt, :], AF.Square)
                for tn in range(2):
                    sl = slice(tn * 512, (tn + 1) * 512)
                    nc.tensor.matmul(sum_ps[tn][:], eseg_sb[:, jt, :],
                                     src_t[:, jt, sl],
                                     start=(jt == 0), stop=(jt == ND - 1))
                    nc.tensor.matmul(ssq_ps[tn][:], eseg_sb[:, jt, :],
                                     sq[:, sl],
                                     start=(jt == 0), stop=(jt == ND - 1))
            # A = rstd, B = -mean*rstd  in [16, T] (bf16 for matmul use)
            muf = small.tile([16, T], F32, tag="muf", name=f"mu{which}")
            varf = small.tile([16, T], F32, tag="varf", name=f"var{which}")
            sscr = temps.tile([16, T], F32, tag="rbc", name=f"ss{which}")
            A_sb = small.tile([16, T], BF16, tag="Asb", bufs=2, name=f"A{which}")
            B_sb = small.tile([16, T], BF16, tag="Bsb", bufs=2, name=f"B{which}")
            for tn in range(2):
                sl = slice(tn * 512, (tn + 1) * 512)
                nc.vector.tensor_scalar_mul(muf[:, sl], sum_ps[tn][:], 1.0 / E)
                nc.vector.tensor_scalar_mul(varf[:, sl], ssq_ps[tn][:], 1.0 / E)
            nc.vector.tensor_mul(sscr[:], muf[:], muf[:])
            nc.vector.tensor_sub(varf[:], varf[:], sscr[:])      # var
            nc.scalar.activation(varf[:], varf[:], AF.Sqrt, bias=eps_sb[:16])
            nc.vector.reciprocal(varf[:], varf[:])               # rstd (f32)
            nc.vector.tensor_copy(A_sb[:], varf[:])
            nc.vector.tensor_mul(B_sb[:], muf[:], varf[:])
            nc.vector.tensor_scalar_mul(B_sb[:], B_sb[:], -1.0)  # -mean*rstd
            AB[which] = (A_sb, B_sb)

        for jt in range(ND):
            for which, src_t, wcol in ((0, qT, wq_sb), (1, kT, wk_sb)):
                A_sb, B_sb = AB[which]
                bcA = psA([P, T], f"bcA{which}_{jt}")
                bcB = psA([P, T], f"bcB{which}_{jt}")
                for tn in range(2):
                    sl = slice(tn * 512, (tn + 1) * 512)
                    nc.tensor.matmul(bcA[:, sl], bseg_sb[:, jt, :],
                                     A_sb[:, sl])
                    nc.tensor.matmul(bcB[:, sl], bseg_sb[:, jt, :],
                                     B_sb[:, sl])
                nc.vector.tensor_mul(src_t[:, jt, :], src_t[:, jt, :], bcA[:])
                nc.vector.tensor_add(src_t[:, jt, :], src_t[:, jt, :], bcB[:])
                if apply_qk_weight:
                    nc.vector.tensor_scalar_mul(src_t[:, jt, :],
                                                src_t[:, jt, :], wcol[:])
                # rope (rows with p%64<32; pass rows have CS=1, SN=0)
                swp = psA([P, T], f"swp{which}_{jt}")
                for tn in range(2):
                    sl = slice(tn * 512, (tn + 1) * 512)
                    nc.tensor.matmul(swp[:, sl], pswap_sb[:], src_t[:, jt, sl])
                nc.vector.tensor_mul(src_t[:, jt, :], src_t[:, jt, :], cs_sb[:])
                sws = temps.tile([P, T], F32, tag="scr_f32",
                                 name=f"sws{which}_{jt}")
                nc.vector.tensor_mul(sws[:], swp[:], sn_sb[:])
                nc.vector.tensor_add(src_t[:, jt, :], src_t[:, jt, :], sws[:])

        # ==============================================================
        # Stage 4: attention, two heads (one q/k row-half pair) at a time
        # ==============================================================
        denA = small.tile([16, T], F32, tag="denA", name="denA")
        nc.vector.memset(denA[:], 0.0)
        for hp in range(H // 2):
            jc = hp
            o_ps = {}
            for h in (2 * hp, 2 * hp + 1):
                for tn in range(2):
                    o_ps[(h, tn)] = psB([E + 16, 512], f"o{h}_{tn}")
            for tk in range(NT):
                sc = {}
                for h in (2 * hp, 2 * hp + 1):
                    p0 = (h % 2) * E
                    sc[h] = psA([P, T], f"sc{h}_{tk}")
                    for tn in range(2):
                        sl = slice(tn * 512, (tn + 1) * 512)
                        nc.tensor.matmul(sc[h][:, sl],
                                         kT[p0:p0 + E, jc, tk * P:(tk + 1) * P],
                                         qT[p0:p0 + E, jc, sl])
                for h in (2 * hp, 2 * hp + 1):
                    ex = temps.tile([P, T], BF16, tag="exp", bufs=4,
                                    name=f"ex{h}_{tk}")
                    nc.scalar.activation(ex[:], sc[h][:], AF.Exp, scale=0.125)
                    for tn in range(2):
                        sl = slice(tn * 512, (tn + 1) * 512)
                        nc.tensor.matmul(o_ps[(h, tn)][:], v_sb[:, tk, h, :],
                                         ex[:, sl],
                                         start=(tk == 0), stop=(tk == NT - 1))
            # evacuate raw o (bf16); accumulate denominators (row 64+h of
            # each head's psum is its denominator, other rows are zero)
            for h in (2 * hp, 2 * hp + 1):
                p0 = (h % 2) * E
                for tn in range(2):
                    sl = slice(tn * 512, (tn + 1) * 512)
                    nc.vector.tensor_copy(oTn[p0:p0 + E, jc, sl],
                                          o_ps[(h, tn)][0:E, :])
                    nc.vector.tensor_add(denA[:, sl], denA[:, sl],
                                         o_ps[(h, tn)][E:E + 16, :])
        # batched reciprocal of all 16 heads' denominators, then
        # broadcast-multiply into oTn via the bseg trick
        rcpA = small.tile([16, T], BF16, tag="rcpA", name="rcpA")
        with nc.allow_low_precision(reason="bf16 softmax denominators"):
            nc.vector.reciprocal(rcpA[:], denA[:])
        for jt in range(ND):
            bcR = psA([P, T], f"bcR{jt}")
            for tn in range(2):
                sl = slice(tn * 512, (tn + 1) * 512)
                nc.tensor.matmul(bcR[:, sl], bseg_sb[:, jt, :], rcpA[:, sl])
            nc.vector.tensor_mul(oTn[:, jt, :], oTn[:, jt, :], bcR[:])

        # ==============================================================
        # Stage 5: y = (oTn.T @ w_out) * gate
        # ==============================================================
        wo_tiles = []
        for kc in range(ND):
            wo = wstr.tile([P, D], BF16, tag=f"wo{kc}", name=f"wo{kc}")
            for half in range(2):
                wof = wstr.tile([P, 512], F32, tag="wof", bufs=2,
                                name=f"wof{kc}_{half}")
                nc.sync.dma_start(wof[:], wout_e[kc * P:(kc + 1) * P,
                                                 half * 512:(half + 1) * 512])
                nc.scalar.copy(wo[:, half * 512:(half + 1) * 512], wof[:])
            wo_tiles.append(wo)
        for tt in range(NT):
            y_sb = temps.tile([P, D], F32, tag="ysb", name=f"y{tt}")
            for tn in range(2):
                yp = psA([P, 512], f"yp{tt}_{tn}")
                sl = slice(tn * 512, (tn + 1) * 512)
                for kc in range(ND):
                    nc.tensor.matmul(yp[:], oTn[:, kc, tt * P:(tt + 1) * P],
                                     wo_tiles[kc][:, sl],
                                     start=(kc == 0), stop=(kc == ND - 1))
                nc.vector.tensor_mul(y_sb[:, sl], yp[:], gateB[:, sl])
            nc.sync.dma_start(out_e[tt * P:(tt + 1) * P, :], y_sb[:])


# =====================================================================
# Host side
# =====================================================================
_NC_CACHE = {}


def _get_nc(apply_qk_weight: bool):
    key = bool(apply_qk_weight)
    if key not in _NC_CACHE:
        _NC_CACHE[key] = build_nc(key)
    return _NC_CACHE[key]


def _make_consts(position, q_norm_w, k_norm_w):
    cs = np.ones((P, T), np.float32)
    sn = np.zeros((P, T), np.float32)
    cos = position[:, :, 0].T.astype(np.float32)   # [16, T]
    sin = position[:, :, 1].T.astype(np.float32)
    for half in (0, 64):
        for rr in range(32):
            j = rr // 2
            cs[half + rr, :] = cos[j]
            sn[half + rr, :] = sin[j] if (rr % 2 == 1) else -sin[j]
    eseg = np.zeros((P, ND, 16), np.float32)
    bseg = np.zeros((16, ND, P), np.float32)
    for t in range(ND):
        for p in range(P):
            m = 2 * t + p // E
            eseg[p, t, m] = 1.0
            bseg[m, t, p] = 1.0
    pswap = np.zeros((P, P), np.float32)
    for m in range(P):
        if (m % E) < 32:
            pswap[m ^ 1, m] = 1.0
    import ml_dtypes  # noqa: deferred import keeps numpy-only callers fast
    return dict(
        cs_full=cs.astype(ml_dtypes.bfloat16), sn_full=sn.astype(ml_dtypes.bfloat16),
        eseg=eseg.astype(ml_dtypes.bfloat16),
        bseg=bseg.astype(ml_dtypes.bfloat16),
        pswap=pswap.astype(ml_dtypes.bfloat16),
        ident=np.eye(P, dtype=np.float32).astype(ml_dtypes.bfloat16),
        ones_row=np.ones((1, P), np.float32).astype(ml_dtypes.bfloat16),
        wq_col=np.tile(q_norm_w.astype(np.float32), 2).reshape(P, 1),
        wk_col=np.tile(k_norm_w.astype(np.float32), 2).reshape(P, 1),
    )


def kernel(x, time, position, mod_w, mod_b, w_qkv, w_out, q_norm_w, k_norm_w):
    x = np.ascontiguousarray(np.asarray(x, dtype=np.float32))
    time = np.ascontiguousarray(np.asarray(time, dtype=np.float32))
    position = np.asarray(position, dtype=np.float32)
    mod_w = np.ascontiguousarray(np.asarray(mod_w, dtype=np.float32))
    mod_b = np.ascontiguousarray(np.asarray(mod_b, dtype=np.float32))
    w_qkv = np.ascontiguousarray(np.asarray(w_qkv, dtype=np.float32))
    w_out = np.ascontiguousarray(np.asarray(w_out, dtype=np.float32))
    q_norm_w = np.asarray(q_norm_w, dtype=np.float32)
    k_norm_w = np.asarray(k_norm_w, dtype=np.float32)

    apply_w = not (np.all(q_norm_w == 1.0) and np.all(k_norm_w == 1.0))
    nc = _get_nc(apply_w)
    consts = _make_consts(position, q_norm_w, k_norm_w)

    in_maps = [
        dict(x=x[b], time=time[b].reshape(TD), mod_w=mod_w, mod_b=mod_b,
             w_qkv=w_qkv, w_out=w_out, **consts)
        for b in range(B)
    ]
    res = run_bass_kernel_spmd(nc, in_maps, core_ids=list(range(B)))
    out = np.stack([res.results[b]["out"] for b in range(B)], axis=0)
    return out.astype(np.float32)


if __name__ == "__main__":
    nc = build_nc(False)
    print("graph built ok")



# revision 12
# speedup vs baseline: 1.2109x; 1.2109x over previous
"""AdaLN-modulated multi-head attention block on 8 TRN2 NeuronCores.

Shapes (hardcoded): B=8, T=1024, D=1024, H=16 heads, e=64 head dim.
Sharding: pure data-parallel - one batch element per core, weights
replicated, no collectives.

Per-core pipeline ("T" suffix = feature-major [feature, token] layout):
  0. mod = silu(time) @ mod_w + mod_b; broadcast shift/scale+1/gate
  1. h = LN(x)*(1+scale)+shift (token-major, bn_stats); hT = h.T (PE)
  2. qT,kT = (h @ w_qkv[:, :2048]).T   (w_qkv stationary, bf16)
     v     = h @ w_qkv[:, 2048:]       (hT stationary), with a ones
             column per head so attn@v also yields softmax denominators
  3. per-head LN + RoPE on qT,kT in feature-major layout.
     Head sums come from host-precomputed per-head column sums of w_qkv
     (sum_e q = hT . wsum); sums of squares from segment matmuls.
     Q gets the full (x-mu)*rstd; K is only centered (x-mu) and its
     rstd/8 is folded into the exp as a per-key activation scale.
  4. per head: scoresT = kT.T @ qT (bf16); exp on ACT with per-partition
     scale A_k/8; oT = [v|1].T @ expT; normalize rows by broadcast
     reciprocal; result overwrites qT storage
  5. y = (oTn.T @ w_out) * gate  (bf16 out-proj, bf16 output)

Weights (mod_w, w_qkv, w_out) and x are converted to bf16 on the host;
output is written bf16 and upcast on the host.
"""

import sys

try:
    import concourse  # noqa: F401  (provided by the environment, e.g. axon_site)
except ImportError:
    sys.path.append("/opt/trn_rl_repo")

import contextlib

import numpy as np

import concourse.bass as bass
import concourse.mybir as mybir
import concourse.tile as tile
from concourse import bacc
from concourse.bass_utils import run_bass_kernel_spmd

F32 = mybir.dt.float32
BF16 = mybir.dt.bfloat16
AF = mybir.ActivationFunctionType
OP = mybir.AluOpType

B, T, D, TD = 8, 1024, 1024, 1024
H, E = 16, 64
P = 128
NT = T // P          # 8 token tiles
ND = D // P          # 8 feature tiles
EPS = 1e-6
N3 = 3 * D
EV = E + 16          # v columns per head incl. ones block


def build_nc(apply_qk_weight: bool):
    nc = bacc.Bacc("TRN2", target_bir_lowering=False, debug=False, num_devices=8)

    aps = {}
    aps["x"] = nc.dram_tensor("x", [T, D], BF16, kind="ExternalInput").ap()
    aps["time"] = nc.dram_tensor("time", [TD], F32, kind="ExternalInput").ap()
    aps["mod_w"] = nc.dram_tensor("mod_w", [TD, N3], BF16, kind="ExternalInput").ap()
    aps["mod_b"] = nc.dram_tensor("mod_b", [N3], F32, kind="ExternalInput").ap()
    aps["w_qkv"] = nc.dram_tensor("w_qkv", [D, N3], BF16, kind="ExternalInput").ap()
    aps["w_out"] = nc.dram_tensor("w_out", [D, D], BF16, kind="ExternalInput").ap()
    # host-precomputed constants
    aps["cs_full"] = nc.dram_tensor("cs_full", [P, T], BF16, kind="ExternalInput").ap()
    aps["sn_full"] = nc.dram_tensor("sn_full", [P, T], BF16, kind="ExternalInput").ap()
    aps["eseg"] = nc.dram_tensor("eseg", [P, ND, 16], BF16, kind="ExternalInput").ap()
    aps["bseg"] = nc.dram_tensor("bseg", [16, ND, P], BF16, kind="ExternalInput").ap()
    aps["wsum"] = nc.dram_tensor("wsum", [P, ND, 48], BF16, kind="ExternalInput").ap()
    aps["pswap"] = nc.dram_tensor("pswap", [P, P], BF16, kind="ExternalInput").ap()
    aps["ident"] = nc.dram_tensor("ident", [P, P], BF16, kind="ExternalInput").ap()
    aps["ident16f"] = nc.dram_tensor("ident16f", [16, 16], F32, kind="ExternalInput").ap()
    aps["ones_row"] = nc.dram_tensor("ones_row", [1, P], BF16, kind="ExternalInput").ap()
    aps["wq_col"] = nc.dram_tensor("wq_col", [P, 1], F32, kind="ExternalInput").ap()
    aps["wk_col"] = nc.dram_tensor("wk_col", [P, 1], F32, kind="ExternalInput").ap()

    aps["out"] = nc.dram_tensor("out", [T, D], BF16, kind="ExternalOutput").ap()

    with tile.TileContext(nc) as tc:
        _body(nc, tc, aps, apply_qk_weight)
    nc.finalize()
    return nc


def _body(nc, tc, aps, apply_qk_weight):
    x_e, time_e, modw_e = aps["x"], aps["time"], aps["mod_w"]
    modb_e, wqkv_e, wout_e = aps["mod_b"], aps["w_qkv"], aps["w_out"]
    out_e = aps["out"]

    ctx = contextlib.ExitStack()
    with ctx:
        consts = ctx.enter_context(tc.tile_pool(name="consts", bufs=1))
        big = ctx.enter_context(tc.tile_pool(name="big", bufs=1))
        wstr = ctx.enter_context(tc.tile_pool(name="wstr", bufs=1))
        temps = ctx.enter_context(tc.tile_pool(name="temps", bufs=2))
        small = ctx.enter_context(tc.tile_pool(name="small", bufs=1))
        psum = ctx.enter_context(tc.tile_pool(name="psum", bufs=2, space="PSUM"))

        def psA(shape, name):
            return psum.tile(shape, F32, tag="psA", bufs=2, name=name,
                             padded_shape=[P, 1024])

        def psB(shape, name):
            return psum.tile(shape, F32, tag="psB", bufs=4, name=name,
                             padded_shape=[P, 512])

        # ---- constants into SBUF -------------------------------------
        def cload(key, shape, dtype, name):
            t = consts.tile(shape, dtype, tag=name, name=name)
            nc.sync.dma_start(t[:], aps[key])
            return t

        cs_sb = cload("cs_full", [P, T], BF16, "cs_sb")
        sn_sb = cload("sn_full", [P, T], BF16, "sn_sb")
        eseg_sb = cload("eseg", [P, ND, 16], BF16, "eseg_sb")
        bseg_sb = cload("bseg", [16, ND, P], BF16, "bseg_sb")
        wsum_sb = cload("wsum", [P, ND, 48], BF16, "wsum_sb")
        pswap_sb = cload("pswap", [P, P], BF16, "pswap_sb")
        ident_sb = cload("ident", [P, P], BF16, "ident_sb")
        ident16f_sb = cload("ident16f", [16, 16], F32, "ident16f_sb")
        ones_sb = cload("ones_row", [1, P], BF16, "ones_sb")
        wq_sb = cload("wq_col", [P, 1], F32, "wq_sb")
        wk_sb = cload("wk_col", [P, 1], F32, "wk_sb")
        eps_sb = consts.tile([P, 1], F32, tag="eps_sb", name="eps_sb")
        nc.vector.memset(eps_sb[:], EPS)

        # ---- big resident tensors ------------------------------------
        hT = big.tile([P, ND, T], BF16, tag="hT", name="hT")       # 16K/part
        qT = big.tile([P, ND, T], BF16, tag="qT", name="qT")       # 16K
        kT = big.tile([P, ND, T], BF16, tag="kT", name="kT")       # 16K
        v_sb = big.tile([P, NT, H, EV], BF16, tag="v", name="v_sb")  # 20K
        # q|k half of w_qkv resident in bf16 (32K/part); v-weights streamed
        wqk_sb = big.tile([P, ND, 2 * D], BF16, tag="wqk", name="wqk_sb")
        for kc in range(ND):
            nc.sync.dma_start(wqk_sb[:, kc, :], wqkv_e[kc * P:(kc + 1) * P, 0:2 * D])
        oTn = qT   # head rows of qT are dead once that head's scores ran

        # ==============================================================
        # Stage 0: mod = silu(time) @ mod_w + mod_b
        # ==============================================================
        t8 = small.tile([P, TD // P], F32, tag="t8", name="t8")
        nc.sync.dma_start(t8[:], time_e.rearrange("(o p) -> p o", p=P))
        silu8 = small.tile([P, TD // P], BF16, tag="silu8", name="silu8")
        nc.scalar.activation(silu8[:], t8[:], AF.Silu)

        # three groups: shift, scale, gate - each [1, 1024], staged then
        # broadcast to [128, 1024]
        bc_dst = {}
        for g, gname in enumerate(("shiftB", "scale1B", "gateB")):
            mrowf = temps.tile([1, D], F32, tag="rbc", bufs=1, name=f"mrowf{g}")
            nc.sync.dma_start(mrowf[:], modb_e[None, g * D:(g + 1) * D])
            mrow = small.tile([1, D], BF16, tag="mrow", bufs=1, name=f"mrow{g}")
            for n2 in range(2):
                col0 = g * D + n2 * 512
                mp = psB([1, 512], f"modp{g}_{n2}")
                for kc in range(TD // P):
                    mw = wstr.tile([P, 512], BF16, tag="modw", bufs=4,
                                   name=f"mw{g}_{n2}_{kc}")
                    nc.sync.dma_start(mw[:], modw_e[kc * P:(kc + 1) * P,
                                                    col0:col0 + 512])
                    nc.tensor.matmul(mp[:], silu8[:, kc:kc + 1], mw[:],
                                     start=(kc == 0), stop=(kc == TD // P - 1))
                # mrowf holds the bias slice; add the matmul result (bf16 out)
                sl = slice(n2 * 512, (n2 + 1) * 512)
                nc.vector.tensor_add(mrow[:, sl], mrowf[:, sl], mp[:])
            if gname == "scale1B":
                nc.vector.tensor_scalar_add(mrow[:], mrow[:], 1.0)
            if gname == "gateB":
                dst = consts.tile([P, D], BF16, tag=gname, name=gname)
                for n2 in range(2):
                    sl = slice(n2 * 512, (n2 + 1) * 512)
                    bp = psA([P, 512], f"bc{g}_{n2}")
                    nc.tensor.matmul(bp[:], ones_sb[:], mrow[:, sl])
                    nc.vector.tensor_copy(dst[:, sl], bp[:])
                bc_dst[gname] = dst
            else:
                # transpose the [1, D] row into per-feature columns [P, ND]
                col = consts.tile([P, ND], F32, tag=f"col{g}", name=f"col{g}")
                cp = psum.tile([P, 2 * ND], BF16, tag="psB", bufs=4,
                               name=f"colp{g}", padded_shape=[P, 512])
                for dc in range(ND):
                    nc.tensor.transpose(cp[:, 2 * dc:2 * dc + 1],
                                        mrow[:, dc * P:(dc + 1) * P],
                                        ident_sb[0:1, 0:1])
                nc.vector.tensor_copy(col[:], cp[:].rearrange(
                    "p (d two) -> p d two", two=2)[:, :, 0])
                bc_dst[gname] = col
        shcol, sc1col, gateB = bc_dst["shiftB"], bc_dst["scale1B"], bc_dst["gateB"]

        # ==============================================================
        # Stage 1: h = LN(x)*(scale+1)+shift ; hT = h.T
        # ==============================================================
        for tt in range(NT):
            xt = temps.tile([P, D], BF16, tag="xt", name=f"xt{tt}")
            nc.sync.dma_start(xt[:], x_e[tt * P:(tt + 1) * P, :])
            st = small.tile([P, 2, 6], F32, tag="bnst", bufs=2, name=f"st{tt}")
            nc.vector.bn_stats(st[:, 0, :], xt[:, 0:512])
            nc.vector.bn_stats(st[:, 1, :], xt[:, 512:1024])
            mv = small.tile([P, 2], F32, tag="bnmv", bufs=2, name=f"mv{tt}")
            nc.vector.bn_aggr(mv[:], st[:])
            sd = small.tile([P, 1], F32, tag="sd", bufs=2, name=f"sd{tt}")
            nc.scalar.activation(sd[:], mv[:, 1:2], AF.Sqrt, bias=eps_sb[:])
            rstd = small.tile([P, 1], F32, tag="rstd", bufs=2, name=f"rstd{tt}")
            nc.vector.reciprocal(rstd[:], sd[:])
            nmr = small.tile([P, 1], F32, tag="nmr", bufs=2, name=f"nmr{tt}")
            nc.vector.tensor_mul(nmr[:], mv[:, 0:1], rstd[:])
            nc.vector.tensor_scalar_mul(nmr[:], nmr[:], -1.0)

            xn = temps.tile([P, D], BF16, tag="xnb", name=f"xn{tt}")
            nc.vector.tensor_scalar(xn[:], xt[:], rstd[:], nmr[:], OP.mult, OP.add)
            for dc in range(ND):
                tp = psum.tile([P, P], BF16, tag="psA", bufs=2,
                               name=f"tr{tt}_{dc}", padded_shape=[P, 1024])
                nc.tensor.transpose(tp[:], xn[:, dc * P:(dc + 1) * P], ident_sb[:])
                nc.vector.tensor_scalar(hT[:, dc, tt * P:(tt + 1) * P], tp[:],
                                        sc1col[:, dc:dc + 1], shcol[:, dc:dc + 1],
                                        OP.mult, OP.add)

        # ==============================================================
        # Stage 2a: v = h @ w_qkv[:, 2048:] (+ ones col per head)
        # ==============================================================
        nc.vector.memset(v_sb[:, :, :, E:EV], 0.0)
        for h in range(H):
            nc.vector.memset(v_sb[:, :, h, E + h], 1.0)
        for nv in range(2):          # 2 blocks of 512 cols
            wv_tiles = []
            for kc in range(ND):
                wv = wstr.tile([P, 512], BF16, tag=f"wv{kc}", name=f"wv{nv}_{kc}")
                nc.sync.dma_start(wv[:], wqkv_e[kc * P:(kc + 1) * P,
                                                2048 + nv * 512:
                                                2048 + (nv + 1) * 512])
                wv_tiles.append(wv)
            for tt in range(NT):
                vp = psA([P, 512], f"vp{nv}_{tt}")
                for kc in range(ND):
                    nc.tensor.matmul(vp[:], hT[:, kc, tt * P:(tt + 1) * P],
                                     wv_tiles[kc][:],
                                     start=(kc == 0), stop=(kc == ND - 1))
                nc.vector.tensor_copy(
                    v_sb[:, tt, nv * 8:(nv + 1) * 8, 0:E],
                    vp[:].rearrange("p (h e) -> p h e", e=E))

        # ==============================================================
        # Stage 2b: qT / kT (feature-major, bf16)
        # ==============================================================
        for jc in range(2 * ND):
            dst = qT if jc < ND else kT
            jd = jc % ND
            for tn in range(2):
                qp = psA([P, 512], f"qp{jc}_{tn}")
                for kc in range(ND):
                    nc.tensor.matmul(qp[:], wqk_sb[:, kc, jc * P:(jc + 1) * P],
                                     hT[:, kc, tn * 512:(tn + 1) * 512],
                                     start=(kc == 0), stop=(kc == ND - 1))
                nc.vector.tensor_copy(dst[:, jd, tn * 512:(tn + 1) * 512], qp[:])

        # ==============================================================
        # Stage 3: per-head stats, then LN/center + RoPE applies.
        # sums via wsum trick (from hT, all 32 head-rows at once);
        # sums of squares via eseg matmuls on qT/kT.
        # ==============================================================
        sum_ps = [psB([48, 512], f"sum_{tn}") for tn in range(2)]
        for kc in range(ND):
            for tn in range(2):
                nc.tensor.matmul(sum_ps[tn][:], wsum_sb[:, kc, :],
                                 hT[:, kc, tn * 512:(tn + 1) * 512],
                                 start=(kc == 0), stop=(kc == ND - 1))
        mu2 = small.tile([48, T], F32, tag="mu2", name="mu2")
        for tn in range(2):
            sl = slice(tn * 512, (tn + 1) * 512)
            nc.vector.tensor_scalar_mul(mu2[:, sl], sum_ps[tn][:], 1.0 / E)

        AB = {}
        akT = consts.tile([P, NT, 16], F32, tag="akT", name="akT")
        for which, src_t in ((0, qT), (1, kT)):
            ssq_ps = [psB([16, 512], f"ssq{which}_{tn}") for tn in range(2)]
            for jt in range(ND):
                sq = temps.tile([P, T], BF16, tag="sqt", bufs=1,
                                name=f"sq{which}_{jt}")
                nc.scalar.activation(sq[:], src_t[:, jt, :], AF.Square)
                for tn in range(2):
                    sl = slice(tn * 512, (tn + 1) * 512)
                    nc.tensor.matmul(ssq_ps[tn][:], eseg_sb[:, jt, :],
                                     sq[:, sl],
                                     start=(jt == 0), stop=(jt == ND - 1))
            muf = mu2[32 * which:32 * which + 16, :]
            varf = small.tile([16, T], F32, tag="varf", bufs=2, name=f"var{which}")
            sscr = temps.tile([16, T], F32, tag="sscr", bufs=1, name=f"ss{which}")
            for tn in range(2):
                sl = slice(tn * 512, (tn + 1) * 512)
                nc.vector.tensor_scalar_mul(varf[:, sl], ssq_ps[tn][:], 1.0 / E)
            nc.vector.tensor_mul(sscr[:], muf[:], muf[:])
            nc.vector.tensor_sub(varf[:], varf[:], sscr[:])      # var
            nc.scalar.activation(varf[:], varf[:], AF.Sqrt, bias=eps_sb[:16])
            nc.vector.reciprocal(varf[:], varf[:])               # rstd (f32)
            B_sb = small.tile([16, T], BF16, tag="Bsb", bufs=2, name=f"B{which}")
            if which == 0:
                # q: full LN -> A = rstd, B = -mean*rstd
                A_sb = small.tile([16, T], BF16, tag="Asb", bufs=1, name="A0")
                nc.vector.tensor_copy(A_sb[:], varf[:])
                nc.vector.tensor_mul(B_sb[:], muf[:], varf[:])
                nc.vector.tensor_scalar_mul(B_sb[:], B_sb[:], -1.0)
                AB[which] = (A_sb, B_sb)
            else:
                # k: only centered on-tensor; rstd/8 folded into exp scale.
                # akT[p, tk, h] = rstd_k[h, tk*128+p] / 8 via PE transposes
                ak8 = small.tile([16, T], F32, tag="ak8", name="ak8")
                nc.vector.tensor_scalar_mul(ak8[:], varf[:], 0.125)
                for tk in range(NT):
                    ap_ = psum.tile([P, 16], F32, tag="psB", bufs=4,
                                    name=f"akp{tk}", padded_shape=[P, 512])
                    nc.tensor.transpose(ap_[:], ak8[:, tk * P:(tk + 1) * P],
                                        ident16f_sb[:])
                    nc.vector.tensor_copy(akT[:, tk, :], ap_[:])
                nc.vector.tensor_scalar_mul(B_sb[:], muf[:], -1.0)
                AB[which] = (None, B_sb)

        for jt in range(ND):
            for which, src_t, wcol in ((0, qT, wq_sb), (1, kT, wk_sb)):
                A_sb, B_sb = AB[which]
                bcB = psA([P, T], f"bcB{which}_{jt}")
                for tn in range(2):
                    sl = slice(tn * 512, (tn + 1) * 512)
                    nc.tensor.matmul(bcB[:, sl], bseg_sb[:, jt, :],
                                     B_sb[:, sl])
                if which == 0:
                    bcA = psA([P, T], f"bcA{which}_{jt}")
                    for tn in range(2):
                        sl = slice(tn * 512, (tn + 1) * 512)
                        nc.tensor.matmul(bcA[:, sl], bseg_sb[:, jt, :],
                                         A_sb[:, sl])
                    nc.vector.tensor_mul(src_t[:, jt, :], src_t[:, jt, :], bcA[:])
                nc.vector.tensor_add(src_t[:, jt, :], src_t[:, jt, :], bcB[:])
                if apply_qk_weight:
                    nc.vector.tensor_scalar_mul(src_t[:, jt, :],
                                                src_t[:, jt, :], wcol[:])
                # rope (rows with p%64<32; pass rows have CS=1, SN=0)
                swp = psA([P, T], f"swp{which}_{jt}")
                for tn in range(2):
                    sl = slice(tn * 512, (tn + 1) * 512)
                    nc.tensor.matmul(swp[:, sl], pswap_sb[:], src_t[:, jt, sl])
                nc.vector.tensor_mul(src_t[:, jt, :], src_t[:, jt, :], cs_sb[:])
                sws = temps.tile([P, T], BF16, tag="scr_sws",
                                 name=f"sws{which}_{jt}")
                nc.vector.tensor_mul(sws[:], swp[:], sn_sb[:])
                nc.vector.tensor_add(src_t[:, jt, :], src_t[:, jt, :], sws[:])

        # ==============================================================
        # Stage 4: attention, two heads (one q/k row-half pair) at a time
        # ==============================================================
        denA = small.tile([16, T], F32, tag="denA", name="denA")
        nc.vector.memset(denA[:], 0.0)
        for hp in range(H // 2):
            jc = hp
            o_ps = {}
            for h in (2 * hp, 2 * hp + 1):
                for tn in range(2):
                    o_ps[(h, tn)] = psB([EV, 512], f"o{h}_{tn}")
            for tk in range(NT):
                sc = {}
                for h in (2 * hp, 2 * hp + 1):
                    p0 = (h % 2) * E
                    sc[h] = psA([P, T], f"sc{h}_{tk}")
                    for tn in range(2):
                        sl = slice(tn * 512, (tn + 1) * 512)
                        nc.tensor.matmul(sc[h][:, sl],
                                         kT[p0:p0 + E, jc, tk * P:(tk + 1) * P],
                                         qT[p0:p0 + E, jc, sl])
                for h in (2 * hp, 2 * hp + 1):
                    ex = temps.tile([P, T], BF16, tag="exp", bufs=4,
                                    name=f"ex{h}_{tk}")
                    nc.scalar.activation(ex[:], sc[h][:], AF.Exp,
                                         scale=akT[:, tk, h:h + 1])
                    for tn in range(2):
                        sl = slice(tn * 512, (tn + 1) * 512)
                        nc.tensor.matmul(o_ps[(h, tn)][:], v_sb[:, tk, h, :],
                                         ex[:, sl],
                                         start=(tk == 0), stop=(tk == NT - 1))
            # evacuate raw o (bf16); accumulate denominators (row 64+h of
            # each head's psum is its denominator, other rows are zero)
            for h in (2 * hp, 2 * hp + 1):
                p0 = (h % 2) * E
                for tn in range(2):
                    sl = slice(tn * 512, (tn + 1) * 512)
                    nc.vector.tensor_copy(oTn[p0:p0 + E, jc, sl],
                                          o_ps[(h, tn)][0:E, :])
                    nc.vector.tensor_add(denA[:, sl], denA[:, sl],
                                         o_ps[(h, tn)][E:EV, :])
        # batched reciprocal of all 16 heads' denominators, then
        # broadcast-multiply into oTn via the bseg trick
        rcpA = small.tile([16, T], BF16, tag="rcpA", name="rcpA")
        with nc.allow_low_precision(reason="bf16 softmax denominators"):
            nc.vector.reciprocal(rcpA[:], denA[:])
        for jt in range(ND):
            bcR = psA([P, T], f"bcR{jt}")
            for tn in range(2):
                sl = slice(tn * 512, (tn + 1) * 512)
                nc.tensor.matmul(bcR[:, sl], bseg_sb[:, jt, :], rcpA[:, sl])
            nc.vector.tensor_mul(oTn[:, jt, :], oTn[:, jt, :], bcR[:])

        # ==============================================================
        # Stage 5: y = (oTn.T @ w_out) * gate
        # ==============================================================
        wo_tiles = []
        for kc in range(ND):
            wo = wstr.tile([P, D], BF16, tag=f"wo{kc}", name=f"wo{kc}")
            nc.sync.dma_start(wo[:], wout_e[kc * P:(kc + 1) * P, :])
            wo_tiles.append(wo)
        for tt in range(NT):
            y_sb = temps.tile([P, D], BF16, tag="ysb", name=f"y{tt}")
            for tn in range(2):
                yp = psA([P, 512], f"yp{tt}_{tn}")
                sl = slice(tn * 512, (tn + 1) * 512)
                for kc in range(ND):
                    nc.tensor.matmul(yp[:], oTn[:, kc, tt * P:(tt + 1) * P],
                                     wo_tiles[kc][:, sl],
                                     start=(kc == 0), stop=(kc == ND - 1))
                nc.vector.tensor_mul(y_sb[:, sl], yp[:], gateB[:, sl])
            nc.sync.dma_start(out_e[tt * P:(tt + 1) * P, :], y_sb[:])


# =====================================================================
# Host side
# =====================================================================
_NC_CACHE = {}


def _get_nc(apply_qk_weight: bool):
    key = bool(apply_qk_weight)
    if key not in _NC_CACHE:
        _NC_CACHE[key] = build_nc(key)
    return _NC_CACHE[key]


def _make_consts(position, q_norm_w, k_norm_w):
    cs = np.ones((P, T), np.float32)
    sn = np.zeros((P, T), np.float32)
    cos = position[:, :, 0].T.astype(np.float32)   # [16, T]
    sin = position[:, :, 1].T.astype(np.float32)
    for half in (0, 64):
        for rr in range(32):
            j = rr // 2
            cs[half + rr, :] = cos[j]
            sn[half + rr, :] = sin[j] if (rr % 2 == 1) else -sin[j]
    eseg = np.zeros((P, ND, 16), np.float32)
    bseg = np.zeros((16, ND, P), np.float32)
    for t in range(ND):
        for p in range(P):
            m = 2 * t + p // E
            eseg[p, t, m] = 1.0
            bseg[m, t, p] = 1.0
    pswap = np.zeros((P, P), np.float32)
    for m in range(P):
        if (m % E) < 32:
            pswap[m ^ 1, m] = 1.0
    import ml_dtypes  # noqa: deferred import keeps numpy-only callers fast
    return dict(
        cs_full=cs.astype(ml_dtypes.bfloat16), sn_full=sn.astype(ml_dtypes.bfloat16),
        eseg=eseg.astype(ml_dtypes.bfloat16),
        bseg=bseg.astype(ml_dtypes.bfloat16),
        pswap=pswap.astype(ml_dtypes.bfloat16),
        ident=np.eye(P, dtype=np.float32).astype(ml_dtypes.bfloat16),
        ident16f=np.eye(16, dtype=np.float32),
        ones_row=np.ones((1, P), np.float32).astype(ml_dtypes.bfloat16),
        wq_col=np.tile(q_norm_w.astype(np.float32), 2).reshape(P, 1),
        wk_col=np.tile(k_norm_w.astype(np.float32), 2).reshape(P, 1),
    )


def _bf16_weights(mod_w, w_qkv, w_out):
    import ml_dtypes
    # per-head column sums of w_qkv (q and k blocks) for mean stats:
    # wsum[d, m] = sum_e w_qkv[d, m*64+e] (m<16: q heads; m>=16: k heads)
    ws32 = w_qkv[:, :2 * D].reshape(D, 32, E).sum(axis=2)    # [D, 32]
    wsum = np.zeros((D, 48), np.float32)
    wsum[:, 0:16] = ws32[:, 0:16]       # q heads
    wsum[:, 32:48] = ws32[:, 16:32]     # k heads (32-partition aligned)
    return dict(
        mod_w=np.ascontiguousarray(mod_w.astype(ml_dtypes.bfloat16)),
        w_qkv=np.ascontiguousarray(w_qkv.astype(ml_dtypes.bfloat16)),
        w_out=np.ascontiguousarray(w_out.astype(ml_dtypes.bfloat16)),
        wsum=np.ascontiguousarray(
            wsum.reshape(ND, P, 48).transpose(1, 0, 2).astype(ml_dtypes.bfloat16)),
    )


def _bf16_x(xb):
    import ml_dtypes
    return np.ascontiguousarray(xb.astype(ml_dtypes.bfloat16))


def kernel(x, time, position, mod_w, mod_b, w_qkv, w_out, q_norm_w, k_norm_w):
    x = np.asarray(x, dtype=np.float32)
    time = np.ascontiguousarray(np.asarray(time, dtype=np.float32))
    position = np.asarray(position, dtype=np.float32)
    mod_w = np.asarray(mod_w, dtype=np.float32)
    mod_b = np.ascontiguousarray(np.asarray(mod_b, dtype=np.float32))
    w_qkv = np.asarray(w_qkv, dtype=np.float32)
    w_out = np.asarray(w_out, dtype=np.float32)
    q_norm_w = np.asarray(q_norm_w, dtype=np.float32)
    k_norm_w = np.asarray(k_norm_w, dtype=np.float32)

    apply_w = not (np.all(q_norm_w == 1.0) and np.all(k_norm_w == 1.0))
    nc = _get_nc(apply_w)
    consts = _make_consts(position, q_norm_w, k_norm_w)
    wts = _bf16_weights(mod_w, w_qkv, w_out)

    in_maps = [
        dict(x=_bf16_x(x[b]), time=time[b].reshape(TD), mod_b=mod_b,
             **wts, **consts)
        for b in range(B)
    ]
    res = run_bass_kernel_spmd(nc, in_maps, core_ids=list(range(B)))
    out = np.stack([res.results[b]["out"] for b in range(B)], axis=0)
    return out.astype(np.float32)


if __name__ == "__main__":
    nc = build_nc(False)
    print("graph built ok")
